# revision 24
# baseline (speedup 1.0000x reference)
import os, sys
import numpy as np

sys.path.insert(0, "/opt/trn_rl_repo")

from concourse import bass, bacc, bass_utils
from concourse import mybir
from concourse.tile import TileContext

F32 = mybir.dt.float32
F32R = mybir.dt.float32r
F16 = mybir.dt.float16
ALU = mybir.AluOpType
ACTF = mybir.ActivationFunctionType

A = 32          # in_maps
B = 32          # out_maps
C = 16          # atoms
H = 64
W = 64
NCORES = 8
ROWS = H // NCORES
NPOS = ROWS * W             # 512 positions per core
NCHUNK = 128
NCH = NPOS // NCHUNK        # 4 chunks
CB = C * B                  # 512
EPS = 1e-4
G = 14                      # a-values packed per partition-stride group
J = 3                       # ceil(A/G)
NP_IN = 9 * G               # 126 partitions for patch/weight tensors

_CACHE = {}


def _build_nc(num_routes: int):
    nc = bacc.Bacc(None, target_bir_lowering=False)

    patches_h_d = nc.declare_dram_parameter("patches_h", [9, A, NPOS], F32R, isOutput=False)
    patches_l_d = nc.declare_dram_parameter("patches_l", [9, A, NPOS], F32R, isOutput=False)
    wv_h_d = nc.declare_dram_parameter("wv_h", [9, A, CB], F32R, isOutput=False)
    wv_l_d = nc.declare_dram_parameter("wv_l", [9, A, CB], F32R, isOutput=False)
    out_d = nc.declare_dram_parameter("out", [NPOS, CB], F32, isOutput=True)

    with TileContext(nc) as tc:
        with (
            tc.tile_pool(name="const", bufs=1) as cpool,
            tc.tile_pool(name="psum", bufs=6, space="PSUM") as pp,
            tc.tile_pool(name="p1psum", bufs=2, space="PSUM") as pp1,
            tc.tile_pool(name="v1", bufs=1) as v1pool,
            tc.tile_pool(name="v2", bufs=1) as v2pool,
            tc.tile_pool(name="work", bufs=1) as wk,
            tc.tile_pool(name="small", bufs=1) as sm,
        ):

            for k in range(NCH):
                v1 = v1pool.tile([NCHUNK, CB * A], F16)    # [p, (c b a)]
                v2 = v2pool.tile([NCHUNK, CB * A], F32)    # [p, (b a c)]
                p1raw = sm.tile([NCHUNK, CB], F32, tag="p1raw")
                p1ps = pp1.tile([NCHUNK, CB], F32)
                v14 = v1[:].rearrange("p (c b a) -> p c b a", c=C, b=B)
                v24 = v2[:].rearrange("p (b a c) -> p b a c", b=B, a=A)

                pch = cpool.tile([9, A * NCHUNK], F32R, tag="pch")
                pcl = cpool.tile([9, A * NCHUNK], F32R, tag="pcl")
                ksl = slice(k * NCHUNK, (k + 1) * NCHUNK)
                nc.sync.dma_start(out=pch[:].rearrange("q (a n) -> q a n", a=A),
                                  in_=patches_h_d.ap()[:, :, ksl])
                nc.sync.dma_start(out=pcl[:].rearrange("q (a n) -> q a n", a=A),
                                  in_=patches_l_d.ap()[:, :, ksl])
                pch3 = pch[:].rearrange("q (a n) -> q a n", a=A)
                pcl3 = pcl[:].rearrange("q (a n) -> q a n", a=A)
                QA = 8
                for qa in range(A // QA):
                    wqh = cpool.tile([9, QA * CB], F32R, tag="wqh")
                    wql = cpool.tile([9, QA * CB], F32R, tag="wql")
                    asl = slice(qa * QA, (qa + 1) * QA)
                    nc.sync.dma_start(out=wqh[:].rearrange("q (a n) -> q a n", a=QA),
                                      in_=wv_h_d.ap()[:, asl, :])
                    nc.sync.dma_start(out=wql[:].rearrange("q (a n) -> q a n", a=QA),
                                      in_=wv_l_d.ap()[:, asl, :])
                    wqh3 = wqh[:].rearrange("q (a n) -> q a n", a=QA)
                    wql3 = wql[:].rearrange("q (a n) -> q a n", a=QA)
                    for al in range(QA):
                        a = qa * QA + al
                        hi_p = pch3[:, a, :]
                        lo_p = pcl3[:, a, :]
                        hi_w = wqh3[:, al, :]
                        lo_w = wql3[:, al, :]
                        nc.tensor.matmul(out=p1ps[:], lhsT=hi_p, rhs=hi_w,
                                         start=(a == 0), stop=False)
                        nc.tensor.matmul(out=p1ps[:], lhsT=hi_p, rhs=lo_w,
                                         start=False, stop=False)
                        nc.tensor.matmul(out=p1ps[:], lhsT=lo_p, rhs=hi_w,
                                         start=False, stop=(a == A - 1))
                        vps = pp.tile([NCHUNK, CB], F32)
                        nc.tensor.matmul(out=vps[:], lhsT=hi_p, rhs=hi_w,
                                         start=True, stop=False)
                        nc.tensor.matmul(out=vps[:], lhsT=hi_p, rhs=lo_w,
                                         start=False, stop=False)
                        nc.tensor.matmul(out=vps[:], lhsT=lo_p, rhs=hi_w,
                                         start=False, stop=True)
                        srcv = vps[:].rearrange("p (c b) -> p c b", c=C)
                        nc.vector.tensor_copy(out=v14[:, :, :, a], in_=srcv)
                        nc.scalar.copy(out=v24[:, :, a, :],
                                       in_=srcv.rearrange("p c b -> p b c"))
                nc.vector.tensor_scalar_mul(out=p1raw[:], in0=p1ps[:], scalar1=1.0 / A)

                logits = sm.tile([NCHUNK, B * A], F32, tag="logits")   # [p, (b a)]
                lsm = sm.tile([NCHUNK, B * A], F32, tag="lsm")
                lsm16 = wk.tile([NCHUNK, B * A], F16, tag="lsm16")
                pcur = sm.tile([NCHUNK, CB], F32, tag="pcur")          # [p, (c b)]
                praw = sm.tile([NCHUNK, CB], F32, tag="praw")
                tmpf = wk.tile([NCHUNK, CB * A // 4], F32, tag="tmpf")
                tmph = tmpf[:].bitcast(F16)

                def squash(p_raw):
                    sq = sm.tile([NCHUNK, B], F32, tag="sq")
                    den = sm.tile([NCHUNK, B], F32, tag="den")
                    fac = sm.tile([NCHUNK, B], F32, tag="fac")
                    p2t = wk.tile([NCHUNK, CB], F32, tag="p2")
                    p2 = p2t[:]
                    nc.vector.tensor_tensor(out=p2, in0=p_raw[:], in1=p_raw[:],
                                            op=ALU.mult)
                    nc.vector.tensor_reduce(
                        out=sq[:],
                        in_=p2.rearrange("p (c b) -> p b c", c=C),
                        axis=mybir.AxisListType.X, op=ALU.add)
                    nc.vector.tensor_scalar_add(out=sq[:], in0=sq[:], scalar1=EPS)
                    nc.scalar.activation(out=den[:], in_=sq[:], func=ACTF.Sqrt)
                    nc.vector.tensor_scalar_add(out=fac[:], in0=sq[:], scalar1=1.0)
                    nc.vector.tensor_tensor(out=den[:], in0=den[:], in1=fac[:],
                                            op=ALU.mult)
                    nc.vector.tensor_scalar_add(out=den[:], in0=den[:], scalar1=EPS)
                    nc.vector.reciprocal(out=den[:], in_=den[:])
                    nc.vector.tensor_tensor(out=fac[:], in0=sq[:], in1=den[:],
                                            op=ALU.mult)
                    nc.vector.tensor_tensor(
                        out=pcur[:].rearrange("p (c b) -> p c b", c=C),
                        in0=p_raw[:].rearrange("p (c b) -> p c b", c=C),
                        in1=fac[:].unsqueeze(1).to_broadcast([NCHUNK, C, B]),
                        op=ALU.mult)

                def delta_update(first):
                    # tmpf[p,b,a,c] = V2 * pcur (bcast a); reduce c -> delta [p,(b a)]
                    QB = B // 4
                    pc_bac = pcur[:].rearrange("p (c b) -> p b c", c=C)
                    for h in range(4):
                        bs = slice(h * QB, (h + 1) * QB)
                        nc.vector.tensor_tensor(
                            out=tmpf[:].rearrange("p (b a c) -> p b a c", b=QB, a=A),
                            in0=v24[:, bs, :, :],
                            in1=pc_bac[:, bs, :].unsqueeze(2)
                                .to_broadcast([NCHUNK, QB, A, C]),
                            op=ALU.mult)
                        nc.vector.tensor_reduce(
                            out=logits[:].rearrange("p (b a) -> p b a", b=B)[:, bs, :],
                            in_=tmpf[:].rearrange("p (b a c) -> p b a c", b=QB, a=A),
                            axis=mybir.AxisListType.X, op=ALU.add)
                    if first:
                        nc.vector.tensor_scalar_add(out=logits[:], in0=logits[:],
                                                    scalar1=1.0 / A)
                    else:
                        nc.vector.tensor_tensor(out=logits[:], in0=logits[:],
                                                in1=lsm[:], op=ALU.add)

                def softmax():
                    ssum = sm.tile([NCHUNK, B], F32, tag="ssum")
                    nc.scalar.activation(out=lsm[:], in_=logits[:], func=ACTF.Exp)
                    nc.vector.tensor_reduce(
                        out=ssum[:], in_=lsm[:].rearrange("p (b a) -> p b a", b=B),
                        axis=mybir.AxisListType.X, op=ALU.add)
                    nc.vector.reciprocal(out=ssum[:], in_=ssum[:])
                    nc.vector.tensor_tensor(
                        out=lsm[:].rearrange("p (b a) -> p b a", b=B),
                        in0=lsm[:].rearrange("p (b a) -> p b a", b=B),
                        in1=ssum[:].unsqueeze(2).to_broadcast([NCHUNK, B, A]),
                        op=ALU.mult)
                    nc.scalar.copy(out=lsm16[:], in_=lsm[:])

                def preds_from_lsm():
                    HC = C // 2
                    for h in range(2):
                        cs = slice(h * HC, (h + 1) * HC)
                        nc.vector.tensor_tensor(
                            out=tmph.rearrange("p (c b a) -> p c b a", c=HC, b=B),
                            in0=v14[:, cs, :, :],
                            in1=lsm16[:].rearrange("p (b a) -> p b a", b=B)
                                .unsqueeze(1).to_broadcast([NCHUNK, HC, B, A]),
                            op=ALU.mult)
                        nc.vector.tensor_reduce(
                            out=praw[:].rearrange("p (c b) -> p c b", c=C)[:, cs, :],
                            in_=tmph.rearrange("p (c b a) -> p c b a", c=HC, b=B),
                            axis=mybir.AxisListType.X, op=ALU.add)

                squash(p1raw)
                if num_routes >= 2:
                    delta_update(True)
                    for it in range(2, num_routes + 1):
                        softmax()
                        preds_from_lsm()
                        squash(praw)
                        if it < num_routes:
                            delta_update(False)

                nc.sync.dma_start(out=out_d.ap()[k * NCHUNK:(k + 1) * NCHUNK, :],
                                  in_=pcur[:])

    nc.compile()
    return nc


def kernel(x=None, weights=None, num_routes=3, **kw):
    x = np.asarray(x, dtype=np.float32)
    weights = np.asarray(weights, dtype=np.float32)
    nr = int(num_routes)

    if nr not in _CACHE:
        _CACHE[nr] = _build_nc(nr)
    nc = _CACHE[nr]

    xp = np.zeros((A, H + 2, W + 2), dtype=np.float32)
    xp[:, 1:-1, 1:-1] = x

    f16r = lambda t: t.astype(np.float16).astype(np.float32)
    wvf = np.ascontiguousarray(weights.reshape(9, A, CB))
    wv_hi = f16r(wvf)
    wv_lo = wvf - wv_hi

    in_maps = []
    for core in range(NCORES):
        r0 = core * ROWS
        pat = np.empty((9, A, ROWS, W), dtype=np.float32)
        for dp in range(3):
            for dq in range(3):
                pat[dp * 3 + dq] = xp[:, r0 + dp:r0 + dp + ROWS, dq:dq + W]
        patf = np.ascontiguousarray(pat.reshape(9, A, NPOS))
        pat_hi = f16r(patf)
        in_maps.append({"patches_h": pat_hi, "patches_l": patf - pat_hi,
                        "wv_h": wv_hi, "wv_l": wv_lo})

    res = bass_utils.run_bass_kernel_spmd(nc, in_maps, core_ids=list(range(NCORES)))

    out = np.empty((B, C, H, W), dtype=np.float32)
    for core in range(NCORES):
        o = np.asarray(res.results[core]["out"]).reshape(ROWS, W, C, B)
        out[:, :, core * ROWS:(core + 1) * ROWS, :] = o.transpose(3, 2, 0, 1)
    return out


def profile_once(inputs):
    """Run once with NTFF tracing on core 0 and return HW exec time in ns."""
    x = np.asarray(inputs["x"], dtype=np.float32)
    weights = np.asarray(inputs["weights"], dtype=np.float32)
    nr = int(inputs.get("num_routes", 3))
    if nr not in _CACHE:
        _CACHE[nr] = _build_nc(nr)
    nc = _CACHE[nr]
    xp = np.zeros((A, H + 2, W + 2), dtype=np.float32)
    xp[:, 1:-1, 1:-1] = x
    f16r = lambda t: t.astype(np.float16).astype(np.float32)
    wvf = np.ascontiguousarray(weights.reshape(9, A, CB))
    wv_hi = f16r(wvf); wv_lo = wvf - wv_hi
    in_maps = []
    for core in range(NCORES):
        r0 = core * ROWS
        pat = np.empty((9, A, ROWS, W), dtype=np.float32)
        for dp in range(3):
            for dq in range(3):
                pat[dp * 3 + dq] = xp[:, r0 + dp:r0 + dp + ROWS, dq:dq + W]
        patf = np.ascontiguousarray(pat.reshape(9, A, NPOS))
        pat_hi = f16r(patf)
        in_maps.append({"patches_h": pat_hi, "patches_l": patf - pat_hi,
                        "wv_h": wv_hi, "wv_l": wv_lo})
    res = bass_utils.run_bass_kernel_spmd(nc, in_maps,
                                          core_ids=list(range(NCORES)),
                                          trace=True, trace_cores=[0])
    if res.exec_time_ns is not None:
        return int(res.exec_time_ns)
    raise RuntimeError("no exec_time_ns from trace")


# revision 25
# speedup vs baseline: 1060.7417x; 1060.7417x over previous
import os, sys
import numpy as np

sys.path.insert(0, "/opt/trn_rl_repo")

from concourse import bass, bacc, bass_utils
from concourse import mybir
from concourse.tile import TileContext

F32 = mybir.dt.float32
F32R = mybir.dt.float32r
F16 = mybir.dt.float16
ALU = mybir.AluOpType
ACTF = mybir.ActivationFunctionType

A = 32          # in_maps
B = 32          # out_maps
C = 16          # atoms
H = 64
W = 64
NCORES = 8
ROWS = H // NCORES
NPOS = ROWS * W             # 512 positions per core
NCHUNK = 128
NCH = NPOS // NCHUNK        # 4 chunks
CB = C * B                  # 512
EPS = 1e-4
G = 14                      # a-values packed per partition-stride group
J = 3                       # ceil(A/G)
NP_IN = 9 * G               # 126 partitions for patch/weight tensors

_CACHE = {}


def _build_nc(num_routes: int):
    nc = bacc.Bacc(None, target_bir_lowering=False)

    patches_h_d = nc.declare_dram_parameter("patches_h", [9, A, NPOS], F32R, isOutput=False)
    patches_l_d = nc.declare_dram_parameter("patches_l", [9, A, NPOS], F32R, isOutput=False)
    wv_h_d = nc.declare_dram_parameter("wv_h", [9, A, CB], F32R, isOutput=False)
    wv_l_d = nc.declare_dram_parameter("wv_l", [9, A, CB], F32R, isOutput=False)
    out_d = nc.declare_dram_parameter("out", [NPOS, CB], F32, isOutput=True)

    with TileContext(nc) as tc:
        with (
            tc.tile_pool(name="const", bufs=1) as cpool,
            tc.tile_pool(name="psum", bufs=6, space="PSUM") as pp,
            tc.tile_pool(name="p1psum", bufs=2, space="PSUM") as pp1,
            tc.tile_pool(name="v1", bufs=1) as v1pool,
            tc.tile_pool(name="v2", bufs=1) as v2pool,
            tc.tile_pool(name="work", bufs=1) as wk,
            tc.tile_pool(name="small", bufs=1) as sm,
        ):

            for k in range(NCH):
                v1 = v1pool.tile([NCHUNK, CB * A], F16)    # [p, (c b a)]
                v2 = v2pool.tile([NCHUNK, CB * A], F32)    # [p, (b a c)]
                p1raw = sm.tile([NCHUNK, CB], F32, tag="p1raw")
                p1ps = pp1.tile([NCHUNK, CB], F32)
                v14 = v1[:].rearrange("p (c b a) -> p c b a", c=C, b=B)
                v24 = v2[:].rearrange("p (b a c) -> p b a c", b=B, a=A)

                pch = cpool.tile([9, A * NCHUNK], F32R, tag="pch")
                pcl = cpool.tile([9, A * NCHUNK], F32R, tag="pcl")
                ksl = slice(k * NCHUNK, (k + 1) * NCHUNK)
                nc.sync.dma_start(out=pch[:].rearrange("q (a n) -> q a n", a=A),
                                  in_=patches_h_d.ap()[:, :, ksl])
                nc.sync.dma_start(out=pcl[:].rearrange("q (a n) -> q a n", a=A),
                                  in_=patches_l_d.ap()[:, :, ksl])
                pch3 = pch[:].rearrange("q (a n) -> q a n", a=A)
                pcl3 = pcl[:].rearrange("q (a n) -> q a n", a=A)
                QA = 8
                for qa in range(A // QA):
                    wqh = cpool.tile([9, QA * CB], F32R, tag="wqh")
                    wql = cpool.tile([9, QA * CB], F32R, tag="wql")
                    asl = slice(qa * QA, (qa + 1) * QA)
                    nc.sync.dma_start(out=wqh[:].rearrange("q (a n) -> q a n", a=QA),
                                      in_=wv_h_d.ap()[:, asl, :])
                    nc.sync.dma_start(out=wql[:].rearrange("q (a n) -> q a n", a=QA),
                                      in_=wv_l_d.ap()[:, asl, :])
                    wqh3 = wqh[:].rearrange("q (a n) -> q a n", a=QA)
                    wql3 = wql[:].rearrange("q (a n) -> q a n", a=QA)
                    for al in range(QA):
                        a = qa * QA + al
                        hi_p = pch3[:, a, :]
                        lo_p = pcl3[:, a, :]
                        hi_w = wqh3[:, al, :]
                        lo_w = wql3[:, al, :]
                        nc.tensor.matmul(out=p1ps[:], lhsT=hi_p, rhs=hi_w,
                                         start=(a == 0), stop=False)
                        nc.tensor.matmul(out=p1ps[:], lhsT=hi_p, rhs=lo_w,
                                         start=False, stop=False)
                        nc.tensor.matmul(out=p1ps[:], lhsT=lo_p, rhs=hi_w,
                                         start=False, stop=(a == A - 1))
                        vps = pp.tile([NCHUNK, CB], F32)
                        nc.tensor.matmul(out=vps[:], lhsT=hi_p, rhs=hi_w,
                                         start=True, stop=False)
                        nc.tensor.matmul(out=vps[:], lhsT=hi_p, rhs=lo_w,
                                         start=False, stop=False)
                        nc.tensor.matmul(out=vps[:], lhsT=lo_p, rhs=hi_w,
                                         start=False, stop=True)
                        srcv = vps[:].rearrange("p (c b) -> p c b", c=C)
                        nc.vector.tensor_copy(out=v24[:, :, a, :],
                                              in_=srcv.rearrange("p c b -> p b c"))
                        nc.gpsimd.tensor_copy(
                            out=v14[:, :, :, a],
                            in_=v24[:, :, a, :].rearrange("p b c -> p c b"))
                nc.vector.tensor_scalar_mul(out=p1raw[:], in0=p1ps[:], scalar1=1.0 / A)

                logits = sm.tile([NCHUNK, B * A], F32, tag="logits")   # [p, (b a)]
                lsm = sm.tile([NCHUNK, B * A], F32, tag="lsm")
                lsm16 = wk.tile([NCHUNK, B * A], F16, tag="lsm16")
                pcur = sm.tile([NCHUNK, CB], F32, tag="pcur")          # [p, (c b)]
                praw = sm.tile([NCHUNK, CB], F32, tag="praw")
                tmpf = wk.tile([NCHUNK, CB * A // 4], F32, tag="tmpf")
                tmph = tmpf[:].bitcast(F16)

                def squash(p_raw):
                    sq = sm.tile([NCHUNK, B], F32, tag="sq")
                    den = sm.tile([NCHUNK, B], F32, tag="den")
                    fac = sm.tile([NCHUNK, B], F32, tag="fac")
                    p2t = wk.tile([NCHUNK, CB], F32, tag="p2")
                    p2 = p2t[:]
                    nc.vector.tensor_tensor(out=p2, in0=p_raw[:], in1=p_raw[:],
                                            op=ALU.mult)
                    nc.vector.tensor_reduce(
                        out=sq[:],
                        in_=p2.rearrange("p (c b) -> p b c", c=C),
                        axis=mybir.AxisListType.X, op=ALU.add)
                    nc.vector.tensor_scalar_add(out=sq[:], in0=sq[:], scalar1=EPS)
                    nc.scalar.activation(out=den[:], in_=sq[:], func=ACTF.Sqrt)
                    nc.vector.tensor_scalar_add(out=fac[:], in0=sq[:], scalar1=1.0)
                    nc.vector.tensor_tensor(out=den[:], in0=den[:], in1=fac[:],
                                            op=ALU.mult)
                    nc.vector.tensor_scalar_add(out=den[:], in0=den[:], scalar1=EPS)
                    nc.vector.reciprocal(out=den[:], in_=den[:])
                    nc.vector.tensor_tensor(out=fac[:], in0=sq[:], in1=den[:],
                                            op=ALU.mult)
                    nc.vector.tensor_tensor(
                        out=pcur[:].rearrange("p (c b) -> p c b", c=C),
                        in0=p_raw[:].rearrange("p (c b) -> p c b", c=C),
                        in1=fac[:].unsqueeze(1).to_broadcast([NCHUNK, C, B]),
                        op=ALU.mult)

                def delta_update(first):
                    # tmpf[p,b,a,c] = V2 * pcur (bcast a); reduce c -> delta [p,(b a)]
                    QB = B // 4
                    pc_bac = pcur[:].rearrange("p (c b) -> p b c", c=C)
                    for h in range(4):
                        bs = slice(h * QB, (h + 1) * QB)
                        nc.vector.tensor_tensor(
                            out=tmpf[:].rearrange("p (b a c) -> p b a c", b=QB, a=A),
                            in0=v24[:, bs, :, :],
                            in1=pc_bac[:, bs, :].unsqueeze(2)
                                .to_broadcast([NCHUNK, QB, A, C]),
                            op=ALU.mult)
                        nc.vector.tensor_reduce(
                            out=logits[:].rearrange("p (b a) -> p b a", b=B)[:, bs, :],
                            in_=tmpf[:].rearrange("p (b a c) -> p b a c", b=QB, a=A),
                            axis=mybir.AxisListType.X, op=ALU.add)
                    if first:
                        nc.vector.tensor_scalar_add(out=logits[:], in0=logits[:],
                                                    scalar1=1.0 / A)
                    else:
                        nc.vector.tensor_tensor(out=logits[:], in0=logits[:],
                                                in1=lsm[:], op=ALU.add)

                def softmax():
                    ssum = sm.tile([NCHUNK, B], F32, tag="ssum")
                    nc.scalar.activation(out=lsm[:], in_=logits[:], func=ACTF.Exp)
                    nc.vector.tensor_reduce(
                        out=ssum[:], in_=lsm[:].rearrange("p (b a) -> p b a", b=B),
                        axis=mybir.AxisListType.X, op=ALU.add)
                    nc.vector.reciprocal(out=ssum[:], in_=ssum[:])
                    nc.vector.tensor_tensor(
                        out=lsm[:].rearrange("p (b a) -> p b a", b=B),
                        in0=lsm[:].rearrange("p (b a) -> p b a", b=B),
                        in1=ssum[:].unsqueeze(2).to_broadcast([NCHUNK, B, A]),
                        op=ALU.mult)
                    nc.gpsimd.tensor_copy(out=lsm16[:], in_=lsm[:])

                def preds_from_lsm():
                    HC = C // 2
                    for h in range(2):
                        cs = slice(h * HC, (h + 1) * HC)
                        nc.vector.tensor_tensor(
                            out=tmph.rearrange("p (c b a) -> p c b a", c=HC, b=B),
                            in0=v14[:, cs, :, :],
                            in1=lsm16[:].rearrange("p (b a) -> p b a", b=B)
                                .unsqueeze(1).to_broadcast([NCHUNK, HC, B, A]),
                            op=ALU.mult)
                        nc.vector.tensor_reduce(
                            out=praw[:].rearrange("p (c b) -> p c b", c=C)[:, cs, :],
                            in_=tmph.rearrange("p (c b a) -> p c b a", c=HC, b=B),
                            axis=mybir.AxisListType.X, op=ALU.add)

                squash(p1raw)
                if num_routes >= 2:
                    delta_update(True)
                    for it in range(2, num_routes + 1):
                        softmax()
                        preds_from_lsm()
                        squash(praw)
                        if it < num_routes:
                            delta_update(False)

                nc.sync.dma_start(out=out_d.ap()[k * NCHUNK:(k + 1) * NCHUNK, :],
                                  in_=pcur[:])

    nc.compile()
    return nc


def kernel(x=None, weights=None, num_routes=3, **kw):
    x = np.asarray(x, dtype=np.float32)
    weights = np.asarray(weights, dtype=np.float32)
    nr = int(num_routes)

    if nr not in _CACHE:
        _CACHE[nr] = _build_nc(nr)
    nc = _CACHE[nr]

    xp = np.zeros((A, H + 2, W + 2), dtype=np.float32)
    xp[:, 1:-1, 1:-1] = x

    f16r = lambda t: t.astype(np.float16).astype(np.float32)
    wvf = np.ascontiguousarray(weights.reshape(9, A, CB))
    wv_hi = f16r(wvf)
    wv_lo = wvf - wv_hi

    in_maps = []
    for core in range(NCORES):
        r0 = core * ROWS
        pat = np.empty((9, A, ROWS, W), dtype=np.float32)
        for dp in range(3):
            for dq in range(3):
                pat[dp * 3 + dq] = xp[:, r0 + dp:r0 + dp + ROWS, dq:dq + W]
        patf = np.ascontiguousarray(pat.reshape(9, A, NPOS))
        pat_hi = f16r(patf)
        in_maps.append({"patches_h": pat_hi, "patches_l": patf - pat_hi,
                        "wv_h": wv_hi, "wv_l": wv_lo})

    res = bass_utils.run_bass_kernel_spmd(nc, in_maps, core_ids=list(range(NCORES)))

    out = np.empty((B, C, H, W), dtype=np.float32)
    for core in range(NCORES):
        o = np.asarray(res.results[core]["out"]).reshape(ROWS, W, C, B)
        out[:, :, core * ROWS:(core + 1) * ROWS, :] = o.transpose(3, 2, 0, 1)
    return out


def profile_once(inputs):
    """Run once with NTFF tracing on core 0 and return HW exec time in ns."""
    x = np.asarray(inputs["x"], dtype=np.float32)
    weights = np.asarray(inputs["weights"], dtype=np.float32)
    nr = int(inputs.get("num_routes", 3))
    if nr not in _CACHE:
        _CACHE[nr] = _build_nc(nr)
    nc = _CACHE[nr]
    xp = np.zeros((A, H + 2, W + 2), dtype=np.float32)
    xp[:, 1:-1, 1:-1] = x
    f16r = lambda t: t.astype(np.float16).astype(np.float32)
    wvf = np.ascontiguousarray(weights.reshape(9, A, CB))
    wv_hi = f16r(wvf); wv_lo = wvf - wv_hi
    in_maps = []
    for core in range(NCORES):
        r0 = core * ROWS
        pat = np.empty((9, A, ROWS, W), dtype=np.float32)
        for dp in range(3):
            for dq in range(3):
                pat[dp * 3 + dq] = xp[:, r0 + dp:r0 + dp + ROWS, dq:dq + W]
        patf = np.ascontiguousarray(pat.reshape(9, A, NPOS))
        pat_hi = f16r(patf)
        in_maps.append({"patches_h": pat_hi, "patches_l": patf - pat_hi,
                        "wv_h": wv_hi, "wv_l": wv_lo})
    res = bass_utils.run_bass_kernel_spmd(nc, in_maps,
                                          core_ids=list(range(NCORES)),
                                          trace=True, trace_cores=[0])
    if res.exec_time_ns is not None:
        return int(res.exec_time_ns)
    raise RuntimeError("no exec_time_ns from trace")
